# revision 1
# baseline (speedup 1.0000x reference)
"""MoE routing kernel for Trainium2 (8 NeuronCores, Bass/Tile).

Problem: y = relu(x @ w[idxs] + b[idxs])
  x: [8192, 1024] f32, idxs: [8192] int in [0,16), w: [16, 1024, 1024] f32,
  b: [16, 1024] f32  ->  y: [8192, 1024] f32

Strategy (expert-parallel):
  - Host: bucket tokens by expert. Assign 2 experts per core (the 8 largest
    experts in slot 0, the 8 smallest in slot 1) so every core runs the SAME
    program: segment 0 = [0, B0) tokens for expert A, segment 1 = [B0, T) for
    expert B, with per-slot padding to the max expert population.
  - Host pre-transposes each core's token block to x_t [IN_F, T] so the
    contraction dim (IN_F) lands on SBUF partitions with contiguous DMAs.
  - Device (per core): load x_t, the 2 expert weight matrices and biases into
    SBUF; for each segment / out-feature tile / token block, accumulate 8
    K-tile matmuls into PSUM (out = w_tile.T @ x_t_tile, i.e. y.T layout),
    then a single fused ScalarE activation does bias-add + ReLU on the way
    from PSUM to SBUF; DMA the y.T tile out.
  - Host: transpose back and scatter rows to their original token positions.
"""

import os

import numpy as np

_P = 128
_NCORES = 8
_E = 16
_IN_F = 1024
_OUT_F = 1024
_KT = _IN_F // _P  # 8 contraction tiles
_MT = _OUT_F // _P  # 8 output-feature tiles
_NBLK = 512  # token block (PSUM free dim, fp32 bank limit)

# Compute dtype for x/w on device: "float32" | "bfloat16" | "float32r"
_DTYPE = os.environ.get("MOE_DTYPE", "bfloat16")

_prog_cache: dict = {}
LAST_RESULT = None  # BassKernelResults of the most recent device run


def _build_program(T: int, B0: int, dtype_str: str):
    from contextlib import ExitStack

    import concourse.mybir as mybir
    import concourse.tile as tile
    from concourse import bacc

    key = (T, B0, dtype_str)
    if key in _prog_cache:
        return _prog_cache[key]

    data_dt = {
        "float32": mybir.dt.float32,
        "bfloat16": mybir.dt.bfloat16,
        "float32r": mybir.dt.float32,
    }[dtype_str]
    mm_dt = {
        "float32": mybir.dt.float32,
        "bfloat16": mybir.dt.bfloat16,
        "float32r": mybir.dt.float32r,
    }[dtype_str]

    nc = bacc.Bacc(
        "TRN2", target_bir_lowering=False, debug=False, num_devices=_NCORES
    )

    xt_d = nc.dram_tensor("xt", [_IN_F, T], data_dt, kind="ExternalInput")
    w2_d = nc.dram_tensor("w2", [2, _IN_F, _OUT_F], data_dt, kind="ExternalInput")
    bt_d = nc.dram_tensor("bt", [_P, 2 * _MT], mybir.dt.float32, kind="ExternalInput")
    yt_d = nc.dram_tensor("yt", [_OUT_F, T], mybir.dt.float32, kind="ExternalOutput")

    relu = mybir.ActivationFunctionType.Relu

    with tile.TileContext(nc) as tc, ExitStack() as ctx:
        const = ctx.enter_context(tc.tile_pool(name="const", bufs=1))
        xpool = ctx.enter_context(tc.tile_pool(name="xpool", bufs=1))
        wpool = ctx.enter_context(tc.tile_pool(name="wpool", bufs=2))
        opool = ctx.enter_context(tc.tile_pool(name="opool", bufs=4))
        pspool = ctx.enter_context(tc.tile_pool(name="pspool", bufs=4, space="PSUM"))

        bt = const.tile([_P, 2 * _MT], mybir.dt.float32, name="bt_sb")
        nc.sync.dma_start(bt[:], bt_d.ap())

        # x_t resident in SBUF: [128, KT, T]; panel k holds in_f rows
        # [128k, 128k+128) with tokens contiguous along the free dim.
        xsb = xpool.tile([_P, _KT, T], data_dt, name="xsb")
        for k in range(_KT):
            nc.sync.dma_start(xsb[:, k], xt_d.ap()[k * _P : (k + 1) * _P, :])

        wsb = []
        for s in range(2):
            wt = wpool.tile([_P, _KT, _OUT_F], data_dt, name=f"wsb{s}", tag=f"w{s}")
            for k in range(_KT):
                nc.sync.dma_start(wt[:, k], w2_d.ap()[s, k * _P : (k + 1) * _P, :])
            wsb.append(wt)

        segs = [(0, 0, B0), (1, B0, T)]
        for s, lo, hi in segs:
            for m in range(_MT):
                t0 = lo
                while t0 < hi:
                    nb = min(_NBLK, hi - t0)
                    ps = pspool.tile(
                        [_P, _NBLK], mybir.dt.float32, name="ps", tag="ps"
                    )[:, :nb]
                    for k in range(_KT):
                        lhsT = wsb[s][:, k, m * _P : (m + 1) * _P]
                        rhs = xsb[:, k, t0 : t0 + nb]
                        if mm_dt != data_dt:
                            lhsT = lhsT.bitcast(mm_dt)
                            rhs = rhs.bitcast(mm_dt)
                        nc.tensor.matmul(
                            ps, lhsT, rhs, start=(k == 0), stop=(k == _KT - 1)
                        )
                    ot = opool.tile([_P, _NBLK], mybir.dt.float32, name="ot", tag="ot")[
                        :, :nb
                    ]
                    nc.scalar.activation(
                        ot, ps, relu, bias=bt[:, s * _MT + m : s * _MT + m + 1]
                    )
                    nc.sync.dma_start(yt_d.ap()[m * _P : (m + 1) * _P, t0 : t0 + nb], ot)
                    t0 += nb

    nc.compile()
    _prog_cache[key] = nc
    return nc


def kernel(x, idxs, w, b):
    global LAST_RESULT
    import ml_dtypes
    from concourse.bass_utils import run_bass_kernel_spmd

    x = np.ascontiguousarray(np.asarray(x, dtype=np.float32))
    idxs = np.asarray(idxs).astype(np.int64)
    w = np.ascontiguousarray(np.asarray(w, dtype=np.float32))
    b = np.ascontiguousarray(np.asarray(b, dtype=np.float32))

    n_tok = x.shape[0]
    np_dt = {
        "float32": np.float32,
        "bfloat16": ml_dtypes.bfloat16,
        "float32r": np.float32,
    }[_DTYPE]

    counts = np.bincount(idxs, minlength=_E)
    order = np.argsort(-counts, kind="stable")
    slot0 = order[:_NCORES]  # 8 largest experts
    slot1 = order[_E - 1 : _NCORES - 1 : -1]  # 8 smallest, reversed for pairing
    B0 = max(int(counts[slot0].max()), 1)
    B1 = max(int(counts[slot1].max()), 1)
    T = B0 + B1

    tok_of = [np.nonzero(idxs == e)[0] for e in range(_E)]

    in_maps = []
    metas = []
    for c in range(_NCORES):
        eA, eB = int(slot0[c]), int(slot1[c])
        ta, tb = tok_of[eA], tok_of[eB]
        xt = np.zeros((_IN_F, T), dtype=np_dt)
        xt[:, : len(ta)] = x[ta].T.astype(np_dt)
        xt[:, B0 : B0 + len(tb)] = x[tb].T.astype(np_dt)
        w2 = np.stack([w[eA], w[eB]]).astype(np_dt)
        bt = np.empty((_P, 2 * _MT), np.float32)
        bt[:, :_MT] = b[eA].reshape(_MT, _P).T
        bt[:, _MT:] = b[eB].reshape(_MT, _P).T
        in_maps.append({"xt": xt, "w2": w2, "bt": bt})
        metas.append((ta, tb))

    nc = _build_program(T, B0, _DTYPE)

    trace = os.environ.get("MOE_TRACE", "0") == "1"
    kwargs = {}
    if trace:
        kwargs["trace"] = True
        tdir = os.environ.get("MOE_TRACE_DIR")
        if tdir:
            os.makedirs(tdir, exist_ok=True)
            kwargs["tmpdir"] = tdir

    res = run_bass_kernel_spmd(nc, in_maps, core_ids=list(range(_NCORES)), **kwargs)
    LAST_RESULT = res

    y = np.empty((n_tok, _OUT_F), dtype=np.float32)
    for c in range(_NCORES):
        ta, tb = metas[c]
        yt = res.results[c]["yt"]
        y[ta] = yt[:, : len(ta)].T
        y[tb] = yt[:, B0 : B0 + len(tb)].T
    return y
